# revision 41
# baseline (speedup 1.0000x reference)
"""Bass/Trainium2 kernel for nn_BiGRIL (gnn_message_passing).

Algebraic structure (h == 0, C == 1):
  x1   = where(mask, x, b_fs)
  z    = W0*x1 + W1*m + b_in            (rank-2 in channels)
  zg   = A^T z  ->  W0*xg + W1*mg + b_in*cg   with xg = A^T x1, mg = A^T m,
                                              cg = A^T 1
  v    = M1 z + M2 zg + b_fold          (K=5 matmul + bias via ACT)
  o    = PReLU(v)                       (ScalarE Prelu, bias_v folded in)
  w    = PB^T o                         (pass B)
  rr   = max(w + bias_f + k, k)         (VectorE TS; k-fold absorbs b_o2)
  out  = W_o2 . rr                      (pass C)

v3 "paired-lane" schedule: chunk pair (g, g+64) packs two 512-position
chunks into the 128 psum partitions (lane0 -> 0:64, lane1 -> 64:128):
one K=128 matmul per pair per pass, one ScalarE Prelu per pair, one
VectorE TS per pair.  Startup DMAs are consolidated into a few large
multi-dim-AP transfers; 40 warmup matmuls hold the PE HAM clock at 8/8.

Sharding: data-parallel over batch (B=8 -> 8 cores), no collectives.
"""

import numpy as np
import sys

sys.path.insert(0, "/opt/trn_rl_repo")

B, C, N, T = 8, 1, 1024, 64
H = 64
NT = N * T          # 65536 per-core output elements
CHUNK = 512
NPAIR = 64          # pairs (g, g+64); chunk c=g is lane0, c=g+64 lane1
HALF = NT // 2      # 32768: lane1 position offset
BLK = 4096          # ma tile columns (64 nodes x 64 steps)
NBLK = 8            # blocks per lane

_CACHE = {}


def _fold_weights(W_fs, b_fs, W_in, b_in, W_gc, b_gc, W_lo, b_lo, prelu_a,
                  W_ro, b_ro, W_o1, b_o1, W_o2, b_o2, adj):
    """Host-side weight folding in float64 for accuracy."""
    f8 = np.float64
    W_in, b_in = W_in.astype(f8), b_in.astype(f8)
    W_gc, b_gc = W_gc.astype(f8), b_gc.astype(f8)
    W_lo, b_lo = W_lo.astype(f8), b_lo.astype(f8)
    W_ro, b_ro = W_ro.astype(f8), b_ro.astype(f8)
    W_o1, b_o1 = W_o1.astype(f8), b_o1.astype(f8)
    W_o2, b_o2 = W_o2.astype(f8), b_o2.astype(f8)

    W0 = W_in[:, 0]           # x1 channel  [64]
    W1 = W_in[:, 1]           # mask channel [64]
    Wlo1 = W_lo[:, :H]
    M1 = Wlo1 @ W_gc[:, :H]
    M2 = Wlo1 @ W_gc[:, H:]
    b_fold = Wlo1 @ b_gc + b_lo

    # x1 = u + bfs with u = (x - bfs)*m; fold the +bfs into the bias and
    # the cg coefficient: b_in' = b_in + W0*bfs
    b_in_p = b_in + W0 * float(b_fs[0])
    PA = np.stack([
        M1 @ W0,
        M1 @ W1,
        M2 @ W0,
        M2 @ W1,
        M2 @ b_in_p,
    ])                                     # [5, 64]  lhsT for pass A
    bias_v = M1 @ b_in_p + b_fold          # [64] -> Prelu bias

    w_ro1 = W_ro[0, :H]                    # [64]
    PB = np.outer(w_ro1, W_o1[:, 0])       # [64(h), 64(f)] lhsT for pass B
    bias_f = W_o1[:, 0] * b_ro[0] + b_o1   # [64]

    den = float(np.sum(W_o2[0] ** 2))
    assert abs(den) > 1e-12
    k = float(b_o2[0]) * W_o2[0] / den     # fold b_o2: W_o2.(rr + k) adds b_o2

    cg = adj.astype(f8).sum(axis=0)        # [N] column sums of adj
    cgrep = np.repeat(cg, T)               # [(n,t)] layout n*T + t

    a = float(prelu_a)
    assert 0.0 < a < 1.0

    # pass A stationary: lane0 streams at rows 0:5 -> out 0:64,
    #                    lane1 streams at rows 32:37 -> out 64:128
    paA = np.zeros((128, 128))
    paA[0:5, 0:64] = PA
    paA[32:37, 64:128] = PA
    # pass B stationary: blockdiag(PB, PB) over packed prelu [128, 512]
    pbB = np.zeros((128, 128))
    pbB[0:64, 0:64] = PB
    pbB[64:128, 64:128] = PB
    # pass C stationary: col 0 <- lane0 W_o2, col 32 <- lane1 W_o2
    pcC = np.zeros((128, 64))
    pcC[0:64, 0] = W_o2[0]
    pcC[64:128, 32] = W_o2[0]

    h16 = np.float16
    fp = np.float32
    # consolidated h16 consts [128, 320]: pa | pb | pc
    ch = np.zeros((128, 320))
    ch[:, 0:128] = paA
    ch[:, 128:256] = pbB
    ch[:, 256:320] = pcC
    # consolidated f32 consts [128, 4]: bk | kk | bfs | bias_v (2 lanes)
    cf = np.zeros((128, 4))
    cf[0:64, 0] = bias_f + k
    cf[64:128, 0] = bias_f + k
    cf[0:64, 1] = k
    cf[64:128, 1] = k
    cf[:, 2] = b_fs[0]
    cf[0:64, 3] = bias_v
    cf[64:128, 3] = bias_v

    return dict(
        consts_h=ch.astype(h16),
        consts_f=cf.astype(fp),
        cgrep=cgrep.astype(h16),
        prelu_a=a,
    )


def _build_program(prelu_a):
    import concourse.bass as bass
    import concourse.bacc as bacc
    import concourse.mybir as mybir
    import concourse.tile as tile

    dt = mybir.dt
    f32 = dt.float32
    h16 = dt.float16
    AF = mybir.ActivationFunctionType
    ALU = mybir.AluOpType

    nc = bacc.Bacc("TRN2", target_bir_lowering=False, debug=False,
                   num_devices=B)

    # host-preshuffled: xs[p, nt*64+t] = x[nt*128+p, t]; ms likewise;
    # adjs[p, nt*1024+c] = adj[nt*128+p, c]  (descriptor-efficient loads)
    xs_d = nc.dram_tensor("xs", [128, 512], h16, kind="ExternalInput")
    ms_d = nc.dram_tensor("ms", [128, 512], h16, kind="ExternalInput")
    zr = nc.dram_tensor("zr", [91, BLK], h16, kind="ExternalInput")
    adjs = nc.dram_tensor("adjs", [128, 8192], h16, kind="ExternalInput")
    cgrep = nc.dram_tensor("cgrep", [NT], h16, kind="ExternalInput")
    ch_d = nc.dram_tensor("consts_h", [128, 320], h16, kind="ExternalInput")
    cf_d = nc.dram_tensor("consts_f", [128, 4], f32, kind="ExternalInput")
    out_d = nc.dram_tensor("out", [NT], h16, kind="ExternalOutput")

    MOVA_BUFS = 4
    from contextlib import ExitStack
    with tile.TileContext(nc) as tc, ExitStack() as ctx:
        const = ctx.enter_context(tc.tile_pool(name="const", bufs=1))
        adjp = ctx.enter_context(tc.tile_pool(name="adjp", bufs=1))
        gmovp = ctx.enter_context(tc.tile_pool(name="gmovp", bufs=1))
        movap = ctx.enter_context(tc.tile_pool(name="movap", bufs=1))
        p2p = ctx.enter_context(tc.tile_pool(name="p2p", bufs=4))
        rrp = ctx.enter_context(tc.tile_pool(name="rrp", bufs=4))
        posbp = ctx.enter_context(tc.tile_pool(name="posbp", bufs=3))
        vps = ctx.enter_context(tc.tile_pool(name="vps", bufs=2, space="PSUM"))
        wps = ctx.enter_context(tc.tile_pool(name="wps", bufs=2, space="PSUM"))
        pop = ctx.enter_context(tc.tile_pool(name="pop", bufs=3, space="PSUM"))
        gps = ctx.enter_context(tc.tile_pool(name="gps", bufs=1, space="PSUM"))

        # ---- consolidated constant loads (2 DMAs) ---------------------
        ch_t = const.tile([128, 320], h16)
        cf_t = const.tile([128, 4], f32)
        nc.sync.dma_start(out=ch_t[:], in_=ch_d[:])
        nc.sync.dma_start(out=cf_t[:], in_=cf_d[:])
        pa_t = ch_t[:, 0:128]
        pb_t = ch_t[:, 128:256]
        pc_t = ch_t[:, 256:320]
        bk_t = cf_t[:, 0:1]
        kk_t = cf_t[:, 1:2]
        bfs_t = cf_t[:, 2:3]
        bv_t = cf_t[:, 3:4]

        # ---- ma tiles: 4 persistent [128, 4096] fp16 ------------------
        # rows 0:5   = lane0 streams (u, m, ug, mg, cg)
        # rows 32:37 = lane1 streams (nodes 512:1024)
        # other rows zeroed by DMA from the zr dram tensor (no DVE memsets)
        ma4 = [movap.tile([128, BLK], h16, tag=f"mova{i}", name=f"mova{i}")
               for i in range(MOVA_BUFS)]
        zeng = [nc.sync, nc.gpsimd, nc.scalar, nc.gpsimd]
        for i in range(MOVA_BUFS):
            zeng[i].dma_start(out=ma4[i][5:32, :], in_=zr[0:27, :])
            zeng[(i + 1) % 3].dma_start(out=ma4[i][37:128, :], in_=zr[:, :])

        # ---- x/m: contiguous loads into the combined gmov tile --------
        # gmov[p, nt*64 + t]       = u[node nt*128+p, t]  (after STT)
        # gmov[p, 512 + nt*64 + t] = m[node nt*128+p, t]
        gmov = gmovp.tile([128, 1024], h16, tag="gmov", name="gmov")
        nc.sync.dma_start(out=gmov[:, 0:512], in_=xs_d[:, :])
        nc.sync.dma_start(out=gmov[:, 512:1024], in_=ms_d[:, :])
        # u = (x - bfs)*m  (one contiguous op; +bfs folded into weights)
        gx_cols = gmov[:, 0:512]
        gm_cols = gmov[:, 512:1024]
        nc.vector.scalar_tensor_tensor(
            out=gx_cols, in0=gx_cols, scalar=bfs_t,
            in1=gm_cols, op0=ALU.subtract, op1=ALU.mult)
        # G-pass moving operand for node-tile nt: [u(64 t) | m(64 t)]
        gmh = gmov[:, :].rearrange("p (h q) -> p h q", h=2)

        # ---- adj: 3 contiguous DMAs off the sync queue ----------------
        adjt = adjp.tile([128, 8192], h16, tag="adjt", name="adjt")
        nc.gpsimd.dma_start(out=adjt[:, 0:3072], in_=adjs[:, 0:3072])
        nc.scalar.dma_start(out=adjt[:, 3072:6144], in_=adjs[:, 3072:6144])
        nc.scalar.dma_start(out=adjt[:, 6144:8192], in_=adjs[:, 6144:8192])

        # ---- HAM warmup: keep PE streaming so the clock hits 8/8 ------
        warm_ps = gps.tile([128, 512], f32, tag="psg", name="warm")
        for wi in range(25):
            nc.tensor.matmul(warm_ps[:, 0:128], pa_t, pb_t,
                             start=True, stop=True, skip_group_check=True)

        out3 = out_d[:].rearrange("(l g c) -> g l c", l=2, c=CHUNK)
        gx = [None] * 8
        ma_t = [None] * NBLK
        vps_t = {}
        wps_t = {}
        p2_t = {}
        rr_t = {}
        po_ps = {}

        def emit_g(mt):
            psg = gps.tile([128, 512], f32, tag="psg", name=f"psg{mt}")
            for nt in range(8):
                nc.tensor.matmul(
                    psg[:, 0:128],
                    adjt[:, nt * 1024 + mt * 128:nt * 1024 + (mt + 1) * 128],
                    gmh[:, :, nt * 64:(nt + 1) * 64],
                    start=(nt == 0), stop=(nt == 7))
            g = gmovp.tile([128, 128], h16, tag=f"gxm{mt}", name=f"gxm{mt}")
            nc.scalar.activation(g[:], psg[:, 0:128], AF.Copy,
                                 bias=0.0, scale=1.0)
            gx[mt] = g

        cg2 = cgrep[:].rearrange("(l q) -> l q", l=2)

        def emit_ma_step(blk, step):
            # one slice (2 DMAs) of the 10 gather-DMAs for block `blk`,
            # spread across pair steps to avoid queue bursts.
            mt0, mt1 = blk // 2, 4 + blk // 2
            p0 = (blk % 2) * 64
            ma = ma4[blk % MOVA_BUFS]
            c0, c1 = mt0 * 64, mt1 * 64
            if step == 0:
                nc.sync.dma_start(out=ma[0:1, :],
                                  in_=gmov[p0:p0 + 64, c0:c0 + 64])
                nc.gpsimd.dma_start(out=ma[32:33, :],
                                    in_=gmov[p0:p0 + 64, c1:c1 + 64])
            elif step == 1:
                nc.sync.dma_start(out=ma[1:2, :],
                                  in_=gmov[p0:p0 + 64, 512 + c0:512 + c0 + 64])
                nc.gpsimd.dma_start(out=ma[33:34, :],
                                    in_=gmov[p0:p0 + 64, 512 + c1:512 + c1 + 64])
            elif step == 2:
                nc.gpsimd.dma_start(out=ma[2:3, :],
                                    in_=gx[mt0][p0:p0 + 64, 0:64])
                nc.sync.dma_start(out=ma[3:4, :],
                                  in_=gx[mt0][p0:p0 + 64, 64:128])
            elif step == 3:
                nc.gpsimd.dma_start(out=ma[34:35, :],
                                    in_=gx[mt1][p0:p0 + 64, 0:64])
                nc.sync.dma_start(out=ma[35:36, :],
                                  in_=gx[mt1][p0:p0 + 64, 64:128])
            elif step == 4:
                nc.gpsimd.dma_start(out=ma[4:5, :],
                                    in_=cg2[0:1, blk * BLK:(blk + 1) * BLK])
                nc.sync.dma_start(out=ma[36:37, :],
                                  in_=cg2[1:2, blk * BLK:(blk + 1) * BLK])
            if step == 4:
                ma_t[blk] = ma

        def st_a(g):
            blk, j = g // 8, g % 8
            if g == 0:
                # G tiles for blocks 0-3 (mt 0,4 then 1,5), block-0 gathers,
                # and a second warmup burst to bridge the gather wait
                emit_g(0)
                emit_g(4)
                for s in range(5):
                    emit_ma_step(0, s)
                warm2 = gps.tile([128, 512], f32, tag="psg", name="warm2")
                for wi in range(25):
                    nc.tensor.matmul(warm2[:, 0:128], pa_t, pb_t,
                                     start=True, stop=True,
                                     skip_group_check=True)
                emit_g(1)
                emit_g(5)
                for s in range(5):
                    emit_ma_step(1, s)
            # emit G tiles one full block-pair ahead (entering block 2k
            # prepares tiles (k+1, k+5) for blocks 2k+2 / 2k+3)
            if j == 0 and blk % 2 == 0 and blk // 2 + 1 < 4 and blk > 0:
                emit_g(blk // 2 + 1)
                emit_g(4 + blk // 2 + 1)
            # spread block blk+2's 10 gather-DMAs over pair steps 0..4
            # (two-block lookahead rides out queue head-of-line stalls)
            if blk + 2 < NBLK and j < 5:
                emit_ma_step(blk + 2, j)
            c0 = j * CHUNK
            ps_v = vps.tile([128, 512], f32, tag="v", name=f"v{g}")
            nc.tensor.matmul(ps_v[:], pa_t, ma_t[blk][:, c0:c0 + CHUNK],
                             start=True, stop=True)
            p2 = p2p.tile([128, CHUNK], h16, tag="p2", name=f"p2{g}")
            nc.scalar.activation(p2[:], ps_v[:], AF.Prelu,
                                 bias=bv_t, scale=1.0, alpha=prelu_a)
            vps_t[g] = ps_v
            p2_t[g] = p2

        def st_b(g):
            del vps_t[g]
            ps_w = wps.tile([128, 512], f32, tag="w", name=f"w{g}")
            nc.tensor.matmul(ps_w[:], pb_t, p2_t[g][:],
                             start=True, stop=True)
            del p2_t[g]
            rr = rrp.tile([128, CHUNK], h16, tag="rr", name=f"rr{g}")
            nc.vector.tensor_scalar(
                out=rr[:, :], in0=ps_w[:, :],
                scalar1=bk_t, scalar2=kk_t,
                op0=ALU.add, op1=ALU.max)
            wps_t[g] = ps_w
            rr_t[g] = rr

        def st_c(g):
            del wps_t[g]
            q = g % 2
            if q == 0:
                po_ps[g // 2] = pop.tile([128, 512], f32, tag="po",
                                         name=f"po{g // 2}")
            # pair g outputs: row 64q = lane0 (chunk g), row 64q+32 = lane1
            nc.tensor.matmul(po_ps[g // 2][64 * q:64 * q + 64, :], pc_t,
                             rr_t[g][:, :], start=True, stop=True,
                             tile_position=(0, 64 * q))
            del rr_t[g]
            if q == 1:
                bi = g // 2
                po_sb = posbp.tile([97, 512], h16, tag="po_sb",
                                   name=f"po_sb{bi}")
                if bi % 2 == 0:
                    nc.scalar.activation(po_sb[0:97, :], po_ps[bi][0:97, :],
                                         AF.Identity, bias=0.0, scale=1.0)
                else:
                    nc.vector.tensor_copy(po_sb[0:97, :], po_ps[bi][0:97, :])
                del po_ps[bi]
                # rows (0,32,64,96) = (pair g-1 lane0, g-1 lane1, g l0, g l1)
                # -> one DMA: dram dims (pair: 512, lane: HALF, elem: 1)
                eng = nc.sync if bi % 2 == 0 else nc.gpsimd
                eng.dma_start(out=out3[g - 1:g + 1], in_=po_sb[0:97:32, :])

        for p in range(NPAIR + 4):
            if p < NPAIR:
                st_a(p)
            if 2 <= p < NPAIR + 2:
                st_b(p - 2)
            if p >= 4:
                st_c(p - 4)

    nc.compile()
    return nc


def _get_program(prelu_a):
    key = ("prog", float(prelu_a))
    if key not in _CACHE:
        _CACHE[key] = _build_program(prelu_a)
    return _CACHE[key]


def make_in_maps(x, mask, W_fs, b_fs, W_in, b_in, adj, W_gc, b_gc, W_lo, b_lo,
                 prelu_a, W_ro, b_ro, W_o1, b_o1, W_o2, b_o2):
    x = np.asarray(x, np.float32)
    mask_f = np.asarray(mask, np.float16)
    adj = np.asarray(adj, np.float32)

    folded = _fold_weights(np.asarray(W_fs), np.asarray(b_fs),
                           np.asarray(W_in), np.asarray(b_in),
                           np.asarray(W_gc), np.asarray(b_gc),
                           np.asarray(W_lo), np.asarray(b_lo),
                           float(prelu_a),
                           np.asarray(W_ro), np.asarray(b_ro),
                           np.asarray(W_o1), np.asarray(b_o1),
                           np.asarray(W_o2), np.asarray(b_o2), adj)

    adjs = np.ascontiguousarray(
        adj.astype(np.float16).reshape(8, 128, N).transpose(1, 0, 2)
    ).reshape(128, 8 * N)
    shared = dict(adjs=adjs, cgrep=folded["cgrep"],
                  consts_h=folded["consts_h"], consts_f=folded["consts_f"],
                  zr=np.zeros((91, BLK), np.float16))
    in_maps = []
    for b in range(B):
        m = dict(shared)
        m["xs"] = np.ascontiguousarray(
            x[b, 0].astype(np.float16).reshape(8, 128, T).transpose(1, 0, 2)
        ).reshape(128, 8 * T)
        m["ms"] = np.ascontiguousarray(
            mask_f[b, 0].reshape(8, 128, T).transpose(1, 0, 2)
        ).reshape(128, 8 * T)
        in_maps.append(m)
    return in_maps, folded["prelu_a"]


def kernel(x, mask, W_fs, b_fs, W_in, b_in, adj, W_gc, b_gc, W_lo, b_lo,
           prelu_a, W_ro, b_ro, W_o1, b_o1, W_o2, b_o2):
    in_maps, a = make_in_maps(x, mask, W_fs, b_fs, W_in, b_in, adj, W_gc,
                              b_gc, W_lo, b_lo, prelu_a, W_ro, b_ro, W_o1,
                              b_o1, W_o2, b_o2)
    nc = _get_program(a)

    from concourse.bass_utils import run_bass_kernel_spmd
    res = run_bass_kernel_spmd(nc, in_maps, list(range(B)))

    out = np.empty((B, C, N, T), np.float32)
    for b in range(B):
        out[b, 0] = np.asarray(res.results[b]["out"]).reshape(N, T)
    return out  # fp16 device output upcast to f32 on assignment


# revision 55
# speedup vs baseline: 1.1765x; 1.1765x over previous
"""Bass/Trainium2 kernel for nn_BiGRIL (gnn_message_passing).

Algebraic structure (h == 0, C == 1):
  x1   = where(mask, x, b_fs)
  z    = W0*x1 + W1*m + b_in            (rank-2 in channels)
  zg   = A^T z  ->  W0*xg + W1*mg + b_in*cg   with xg = A^T x1, mg = A^T m,
                                              cg = A^T 1
  v    = M1 z + M2 zg + b_fold          (K=5 matmul + bias via ACT)
  o    = PReLU(v)                       (ScalarE Prelu, bias_v folded in)
  w    = PB^T o                         (pass B)
  rr   = max(w + bias_f + k, k)         (VectorE TS; k-fold absorbs b_o2)
  out  = W_o2 . rr                      (pass C)

v3 "paired-lane" schedule: chunk pair (g, g+64) packs two 512-position
chunks into the 128 psum partitions (lane0 -> 0:64, lane1 -> 64:128):
one K=128 matmul per pair per pass, one ScalarE Prelu per pair, one
VectorE TS per pair.  Startup DMAs are consolidated into a few large
multi-dim-AP transfers; 40 warmup matmuls hold the PE HAM clock at 8/8.

Sharding: data-parallel over batch (B=8 -> 8 cores), no collectives.
"""

import numpy as np
import sys

sys.path.insert(0, "/opt/trn_rl_repo")

B, C, N, T = 8, 1, 1024, 64
H = 64
NT = N * T          # 65536 per-core output elements
CHUNK = 512
NPAIR = 64          # pairs (g, g+64); chunk c=g is lane0, c=g+64 lane1
HALF = NT // 2      # 32768: lane1 position offset
BLK = 4096          # ma tile columns (64 nodes x 64 steps)
NBLK = 8            # blocks per lane

_CACHE = {}


def _fold_weights(W_fs, b_fs, W_in, b_in, W_gc, b_gc, W_lo, b_lo, prelu_a,
                  W_ro, b_ro, W_o1, b_o1, W_o2, b_o2, adj):
    """Host-side weight folding in float64 for accuracy."""
    f8 = np.float64
    W_in, b_in = W_in.astype(f8), b_in.astype(f8)
    W_gc, b_gc = W_gc.astype(f8), b_gc.astype(f8)
    W_lo, b_lo = W_lo.astype(f8), b_lo.astype(f8)
    W_ro, b_ro = W_ro.astype(f8), b_ro.astype(f8)
    W_o1, b_o1 = W_o1.astype(f8), b_o1.astype(f8)
    W_o2, b_o2 = W_o2.astype(f8), b_o2.astype(f8)

    W0 = W_in[:, 0]           # x1 channel  [64]
    W1 = W_in[:, 1]           # mask channel [64]
    Wlo1 = W_lo[:, :H]
    M1 = Wlo1 @ W_gc[:, :H]
    M2 = Wlo1 @ W_gc[:, H:]
    b_fold = Wlo1 @ b_gc + b_lo

    # x1 = u + bfs with u = (x - bfs)*m; fold the +bfs into the bias and
    # the cg coefficient: b_in' = b_in + W0*bfs
    b_in_p = b_in + W0 * float(b_fs[0])
    PA = np.stack([
        M1 @ W0,
        M1 @ W1,
        M2 @ W0,
        M2 @ W1,
        M2 @ b_in_p,
    ])                                     # [5, 64]  lhsT for pass A
    bias_v = M1 @ b_in_p + b_fold          # [64] -> Prelu bias

    w_ro1 = W_ro[0, :H]                    # [64]
    PB = np.outer(w_ro1, W_o1[:, 0])       # [64(h), 64(f)] lhsT for pass B
    bias_f = W_o1[:, 0] * b_ro[0] + b_o1   # [64]

    den = float(np.sum(W_o2[0] ** 2))
    assert abs(den) > 1e-12
    k = float(b_o2[0]) * W_o2[0] / den     # fold b_o2: W_o2.(rr + k) adds b_o2

    cg = adj.astype(f8).sum(axis=0)        # [N] column sums of adj
    cgrep = np.repeat(cg, T)               # [(n,t)] layout n*T + t

    a = float(prelu_a)
    assert 0.0 < a < 1.0

    # pass A stationary: lane0 streams at rows 0:5 -> out 0:64,
    #                    lane1 streams at rows 32:37 -> out 64:128
    paA = np.zeros((128, 128))
    paA[0:5, 0:64] = PA
    paA[32:37, 64:128] = PA
    # pass B stationary: blockdiag(PB, PB) over packed prelu [128, 512]
    pbB = np.zeros((128, 128))
    pbB[0:64, 0:64] = PB
    pbB[64:128, 64:128] = PB
    # pass C stationary: col 0 <- lane0 W_o2, col 32 <- lane1 W_o2
    pcC = np.zeros((128, 64))
    pcC[0:64, 0] = W_o2[0]
    pcC[64:128, 32] = W_o2[0]

    h16 = np.float16
    fp = np.float32
    # consolidated h16 consts [128, 320]: pa | pb | pc
    ch = np.zeros((128, 320))
    ch[:, 0:128] = paA
    ch[:, 128:256] = pbB
    ch[:, 256:320] = pcC
    # consolidated f32 consts [128, 4]: bk | kk | bfs | bias_v (2 lanes)
    cf = np.zeros((128, 4))
    cf[0:64, 0] = bias_f + k
    cf[64:128, 0] = bias_f + k
    cf[0:64, 1] = k
    cf[64:128, 1] = k
    cf[:, 2] = b_fs[0]
    cf[0:64, 3] = bias_v
    cf[64:128, 3] = bias_v

    return dict(
        consts_h=ch.astype(h16),
        consts_f=cf.astype(fp),
        cgrep=cgrep.astype(h16),
        prelu_a=a,
    )


def _build_program(prelu_a):
    import concourse.bass as bass
    import concourse.bacc as bacc
    import concourse.mybir as mybir
    import concourse.tile as tile

    dt = mybir.dt
    f32 = dt.float32
    h16 = dt.float16
    AF = mybir.ActivationFunctionType
    ALU = mybir.AluOpType

    nc = bacc.Bacc("TRN2", target_bir_lowering=False, debug=False,
                   num_devices=B)

    # host-preshuffled: xs[p, nt*64+t] = x[nt*128+p, t]; ms likewise;
    # adjs[p, nt*1024+c] = adj[nt*128+p, c]  (descriptor-efficient loads)
    xs_d = nc.dram_tensor("xs", [128, 512], h16, kind="ExternalInput")
    ms_d = nc.dram_tensor("ms", [128, 512], h16, kind="ExternalInput")
    # xm2 row0 = u = (x-bfs)*m written back from gmov in position order,
    # row1 = m flat (host); per-block stream loads are then cheap
    # [2, 4096] two-descriptor reads
    xm2_d = nc.dram_tensor("xm2", [2, NT], h16, kind="ExternalInput")
    zr = nc.dram_tensor("zr", [91, BLK], h16, kind="ExternalInput")
    adjs = nc.dram_tensor("adjs", [128, 8192], h16, kind="ExternalInput")
    cgrep = nc.dram_tensor("cgrep", [NT], h16, kind="ExternalInput")
    ch_d = nc.dram_tensor("consts_h", [128, 320], h16, kind="ExternalInput")
    cf_d = nc.dram_tensor("consts_f", [128, 4], f32, kind="ExternalInput")
    out_d = nc.dram_tensor("out", [NT], h16, kind="ExternalOutput")

    MOVA_BUFS = 8
    from contextlib import ExitStack
    with tile.TileContext(nc) as tc, ExitStack() as ctx:
        const = ctx.enter_context(tc.tile_pool(name="const", bufs=1))
        adjp = ctx.enter_context(tc.tile_pool(name="adjp", bufs=1))
        gmovp = ctx.enter_context(tc.tile_pool(name="gmovp", bufs=1))
        movap = ctx.enter_context(tc.tile_pool(name="movap", bufs=1))
        p2p = ctx.enter_context(tc.tile_pool(name="p2p", bufs=4))
        rrp = ctx.enter_context(tc.tile_pool(name="rrp", bufs=4))
        posbp = ctx.enter_context(tc.tile_pool(name="posbp", bufs=3))
        vps = ctx.enter_context(tc.tile_pool(name="vps", bufs=2, space="PSUM"))
        wps = ctx.enter_context(tc.tile_pool(name="wps", bufs=2, space="PSUM"))
        pop = ctx.enter_context(tc.tile_pool(name="pop", bufs=3, space="PSUM"))
        gps = ctx.enter_context(tc.tile_pool(name="gps", bufs=1, space="PSUM"))

        # ---- consolidated constant loads (2 DMAs) ---------------------
        ch_t = const.tile([128, 320], h16)
        cf_t = const.tile([128, 4], f32)
        nc.sync.dma_start(out=ch_t[:], in_=ch_d[:])
        nc.sync.dma_start(out=cf_t[:], in_=cf_d[:])
        pa_t = ch_t[:, 0:128]
        pb_t = ch_t[:, 128:256]
        pc_t = ch_t[:, 256:320]
        bk_t = cf_t[:, 0:1]
        kk_t = cf_t[:, 1:2]
        bfs_t = cf_t[:, 2:3]
        bv_t = cf_t[:, 3:4]

        # ---- ma tiles: 8 persistent [128, 4096] fp16, one per block ---
        # rows 0:5   = lane0 streams (xm, m, ug, mg, cg)
        # rows 32:37 = lane1 streams (nodes 512:1024)
        # other rows zeroed by DMA from the zr dram tensor; cg rows static
        ma4 = [movap.tile([128, BLK], h16, tag=f"mova{i}", name=f"mova{i}")
               for i in range(MOVA_BUFS)]

        # u written back to xm2 row 0 piecewise (per node-tile, contiguous
        # in position order).  All xm2[0] readers stay on the gpsimd queue
        # (FIFO) so no cross-queue DRAM ordering is needed.
        u_done = set()

        def emit_u_piece(k):
            if k in u_done:
                return
            u_done.add(k)
            nc.gpsimd.dma_start(out=xm2_d[0:1, k * 8192:(k + 1) * 8192],
                                in_=gmov[:, k * 64:(k + 1) * 64])

        cg2 = cgrep[:].rearrange("(l q) -> l q", l=2)

        def emit_init(blk):
            ma = ma4[blk]
            nc.sync.dma_start(out=ma[5:32, :], in_=zr[0:27, :])
            nc.scalar.dma_start(out=ma[37:128, :], in_=zr[:, :])
            nc.sync.dma_start(out=ma[4:5, :],
                              in_=cg2[0:1, blk * BLK:(blk + 1) * BLK])
            nc.gpsimd.dma_start(out=ma[36:37, :],
                                in_=cg2[1:2, blk * BLK:(blk + 1) * BLK])

        def emit_xm(blk):
            # rows 0:2 = (x*m, m) from the xm2 scratch — single-descriptor
            # DMAs; MUST stay on gpsimd (ordered after the accum DMA)
            ma = ma4[blk]
            nc.gpsimd.dma_start(
                out=ma[0:2, :], in_=xm2_d[:, blk * BLK:(blk + 1) * BLK])
            nc.gpsimd.dma_start(
                out=ma[32:34, :],
                in_=xm2_d[:, HALF + blk * BLK:HALF + (blk + 1) * BLK])

        def emit_gx(blk, lane):
            mt = blk // 2 + 4 * lane
            p0 = (blk % 2) * 64
            ma = ma4[blk]
            r = 2 + 32 * lane
            eng = nc.sync if lane == 0 else nc.gpsimd
            eng.dma_start(out=ma[r:r + 1, :], in_=gx[mt][p0:p0 + 64, 0:64])
            eng.dma_start(out=ma[r + 1:r + 2, :],
                          in_=gx[mt][p0:p0 + 64, 64:128])

        # ---- x/m: contiguous loads into the combined gmov tile --------
        # gmov[p, nt*64 + t]       = u[node nt*128+p, t]  (after STT)
        # gmov[p, 512 + nt*64 + t] = m[node nt*128+p, t]
        gmov = gmovp.tile([128, 1024], h16, tag="gmov", name="gmov")
        nc.sync.dma_start(out=gmov[:, 0:512], in_=xs_d[:, :])
        nc.sync.dma_start(out=gmov[:, 512:1024], in_=ms_d[:, :])
        # u = (x - bfs)*m  (one contiguous op; +bfs folded into weights)
        gx_cols = gmov[:, 0:512]
        gm_cols = gmov[:, 512:1024]
        nc.vector.scalar_tensor_tensor(
            out=gx_cols, in0=gx_cols, scalar=bfs_t,
            in1=gm_cols, op0=ALU.subtract, op1=ALU.mult)
        # G-pass moving operand for node-tile nt: [u(64 t) | m(64 t)]
        gmh = gmov[:, :].rearrange("p (h q) -> p h q", h=2)

        # ---- adj: 3 contiguous DMAs off the sync queue ----------------
        adjt = adjp.tile([128, 8192], h16, tag="adjt", name="adjt")
        nc.gpsimd.dma_start(out=adjt[:, 0:3072], in_=adjs[:, 0:3072])
        nc.scalar.dma_start(out=adjt[:, 3072:6144], in_=adjs[:, 3072:6144])
        nc.scalar.dma_start(out=adjt[:, 6144:8192], in_=adjs[:, 6144:8192])

        # ---- HAM warmup: keep PE streaming so the clock hits 8/8 ------
        warm_ps = gps.tile([128, 512], f32, tag="psg", name="warm")
        for wi in range(25):
            nc.tensor.matmul(warm_ps[:, 0:128], pa_t, pb_t,
                             start=True, stop=True, skip_group_check=True)

        out3 = out_d[:].rearrange("(l g c) -> g l c", l=2, c=CHUNK)
        gx = [None] * 8
        ma_t = [None] * NBLK
        vps_t = {}
        wps_t = {}
        p2_t = {}
        rr_t = {}
        po_ps = {}

        def emit_g(mt):
            psg = gps.tile([128, 512], f32, tag="psg", name=f"psg{mt}")
            for nt in range(8):
                nc.tensor.matmul(
                    psg[:, 0:128],
                    adjt[:, nt * 1024 + mt * 128:nt * 1024 + (mt + 1) * 128],
                    gmh[:, :, nt * 64:(nt + 1) * 64],
                    start=(nt == 0), stop=(nt == 7))
            g = gmovp.tile([128, 128], h16, tag=f"gxm{mt}", name=f"gxm{mt}")
            nc.scalar.activation(g[:], psg[:, 0:128], AF.Copy,
                                 bias=0.0, scale=1.0)
            gx[mt] = g

        cg2 = cgrep[:].rearrange("(l q) -> l q", l=2)

        def emit_ma_step(blk, step):
            # one slice of block `blk`'s prep, spread across pair steps
            if step == 0:
                emit_init(blk)
                emit_u_piece(blk // 2)
                emit_u_piece(4 + blk // 2)
            elif step == 1:
                emit_xm(blk)
            elif step == 2:
                emit_gx(blk, 0)
            elif step == 3:
                emit_gx(blk, 1)
                ma_t[blk] = ma4[blk]

        def st_a(g):
            blk, j = g // 8, g % 8
            if g == 0:
                # G tiles for blocks 0-3 (mt 0,4 then 1,5), block-0/1 prep,
                # and a second warmup burst to bridge the gather wait
                emit_g(0)
                emit_g(4)
                emit_init(0)
                emit_init(1)
                emit_u_piece(0)
                emit_u_piece(4)
                emit_xm(0)
                emit_xm(1)
                warm2 = gps.tile([128, 512], f32, tag="psg", name="warm2")
                for wi in range(25):
                    nc.tensor.matmul(warm2[:, 0:128], pa_t, pb_t,
                                     start=True, stop=True,
                                     skip_group_check=True)
                emit_g(1)
                emit_g(5)
                for b0 in (0, 1):
                    emit_gx(b0, 0)
                    emit_gx(b0, 1)
                    ma_t[b0] = ma4[b0]
            # emit G tiles one full block-pair ahead (entering block 2k
            # prepares tiles (k+1, k+5) for blocks 2k+2 / 2k+3)
            if j == 0 and blk % 2 == 0 and blk // 2 + 1 < 4 and blk > 0:
                emit_g(blk // 2 + 1)
                emit_g(4 + blk // 2 + 1)
            # spread block blk+2's prep DMAs over pair steps 0..3
            # (two-block lookahead rides out queue head-of-line stalls)
            if blk + 2 < NBLK and j < 4:
                emit_ma_step(blk + 2, j)
            c0 = j * CHUNK
            ps_v = vps.tile([128, 512], f32, tag="v", name=f"v{g}")
            nc.tensor.matmul(ps_v[:], pa_t, ma_t[blk][:, c0:c0 + CHUNK],
                             start=True, stop=True)
            p2 = p2p.tile([128, CHUNK], h16, tag="p2", name=f"p2{g}")
            nc.scalar.activation(p2[:], ps_v[:], AF.Prelu,
                                 bias=bv_t, scale=1.0, alpha=prelu_a)
            vps_t[g] = ps_v
            p2_t[g] = p2

        def st_b(g):
            del vps_t[g]
            ps_w = wps.tile([128, 512], f32, tag="w", name=f"w{g}")
            nc.tensor.matmul(ps_w[:], pb_t, p2_t[g][:],
                             start=True, stop=True)
            del p2_t[g]
            rr = rrp.tile([128, CHUNK], h16, tag="rr", name=f"rr{g}")
            nc.vector.tensor_scalar(
                out=rr[:, :], in0=ps_w[:, :],
                scalar1=bk_t, scalar2=kk_t,
                op0=ALU.add, op1=ALU.max)
            wps_t[g] = ps_w
            rr_t[g] = rr

        def st_c(g):
            del wps_t[g]
            q = g % 2
            if q == 0:
                po_ps[g // 2] = pop.tile([128, 512], f32, tag="po",
                                         name=f"po{g // 2}")
            # pair g outputs: row 64q = lane0 (chunk g), row 64q+32 = lane1
            nc.tensor.matmul(po_ps[g // 2][64 * q:64 * q + 64, :], pc_t,
                             rr_t[g][:, :], start=True, stop=True,
                             tile_position=(0, 64 * q))
            del rr_t[g]
            if q == 1:
                bi = g // 2
                po_sb = posbp.tile([97, 512], h16, tag="po_sb",
                                   name=f"po_sb{bi}")
                if bi % 2 == 0:
                    nc.scalar.activation(po_sb[0:97, :], po_ps[bi][0:97, :],
                                         AF.Identity, bias=0.0, scale=1.0)
                else:
                    nc.vector.tensor_copy(po_sb[0:97, :], po_ps[bi][0:97, :])
                del po_ps[bi]
                # rows (0,32,64,96) = (pair g-1 lane0, g-1 lane1, g l0, g l1)
                # -> one DMA: dram dims (pair: 512, lane: HALF, elem: 1)
                eng = nc.sync if bi % 2 == 0 else nc.gpsimd
                eng.dma_start(out=out3[g - 1:g + 1], in_=po_sb[0:97:32, :])

        for p in range(NPAIR + 4):
            if p < NPAIR:
                st_a(p)
            if 2 <= p < NPAIR + 2:
                st_b(p - 2)
            if p >= 4:
                st_c(p - 4)

    nc.compile()
    return nc


def _get_program(prelu_a):
    key = ("prog", float(prelu_a))
    if key not in _CACHE:
        _CACHE[key] = _build_program(prelu_a)
    return _CACHE[key]


def make_in_maps(x, mask, W_fs, b_fs, W_in, b_in, adj, W_gc, b_gc, W_lo, b_lo,
                 prelu_a, W_ro, b_ro, W_o1, b_o1, W_o2, b_o2):
    x = np.asarray(x, np.float32)
    mask_f = np.asarray(mask, np.float16)
    adj = np.asarray(adj, np.float32)

    folded = _fold_weights(np.asarray(W_fs), np.asarray(b_fs),
                           np.asarray(W_in), np.asarray(b_in),
                           np.asarray(W_gc), np.asarray(b_gc),
                           np.asarray(W_lo), np.asarray(b_lo),
                           float(prelu_a),
                           np.asarray(W_ro), np.asarray(b_ro),
                           np.asarray(W_o1), np.asarray(b_o1),
                           np.asarray(W_o2), np.asarray(b_o2), adj)

    adjs = np.ascontiguousarray(
        adj.astype(np.float16).reshape(8, 128, N).transpose(1, 0, 2)
    ).reshape(128, 8 * N)
    shared = dict(adjs=adjs, cgrep=folded["cgrep"],
                  consts_h=folded["consts_h"], consts_f=folded["consts_f"],
                  zr=np.zeros((91, BLK), np.float16))
    in_maps = []
    for b in range(B):
        m = dict(shared)
        xh = x[b, 0].astype(np.float16)
        mh = mask_f[b, 0]
        m["xs"] = np.ascontiguousarray(
            xh.reshape(8, 128, T).transpose(1, 0, 2)).reshape(128, 8 * T)
        m["ms"] = np.ascontiguousarray(
            mh.reshape(8, 128, T).transpose(1, 0, 2)).reshape(128, 8 * T)
        m["xm2"] = np.ascontiguousarray(
            np.stack([np.zeros(NT, np.float16), mh.reshape(-1)]))
        in_maps.append(m)
    return in_maps, folded["prelu_a"]


def kernel(x, mask, W_fs, b_fs, W_in, b_in, adj, W_gc, b_gc, W_lo, b_lo,
           prelu_a, W_ro, b_ro, W_o1, b_o1, W_o2, b_o2):
    in_maps, a = make_in_maps(x, mask, W_fs, b_fs, W_in, b_in, adj, W_gc,
                              b_gc, W_lo, b_lo, prelu_a, W_ro, b_ro, W_o1,
                              b_o1, W_o2, b_o2)
    nc = _get_program(a)

    from concourse.bass_utils import run_bass_kernel_spmd
    res = run_bass_kernel_spmd(nc, in_maps, list(range(B)))

    out = np.empty((B, C, N, T), np.float32)
    for b in range(B):
        out[b, 0] = np.asarray(res.results[b]["out"]).reshape(N, T)
    return out  # fp16 device output upcast to f32 on assignment


# revision 56
# speedup vs baseline: 1.1954x; 1.0160x over previous
"""Bass/Trainium2 kernel for nn_BiGRIL (gnn_message_passing).

Algebraic structure (h == 0, C == 1):
  x1   = where(mask, x, b_fs) = u + bfs,  u = (x - bfs)*m
  z    = W0*x1 + W1*m + b_in            (rank-2 in channels)
  zg   = A^T z  ->  W0*ug + W1*mg + b_in'*cg   with ug = A^T u, mg = A^T m,
                                               cg = A^T 1, b_in' = b_in+W0*bfs
  v    = M1 z + M2 zg + b_fold          (K=5 matmul + bias via ACT Prelu)
  o    = PReLU(v)                       (ScalarE Prelu, bias_v folded in)
  w    = PB^T o                         (pass B)
  rr   = max(w + bias_f + k, k)         (VectorE TS; k-fold absorbs b_o2)
  out  = W_o2 . rr                      (pass C; fp16 out, host upcasts)

Paired-lane schedule: chunk pair (g, g+64) packs two 512-position chunks
into the 128 psum partitions (lane0 -> 0:64, lane1 -> 64:128): one K=128
matmul per pair per pass, one ScalarE Prelu per pair, one VectorE TS per
pair.  Inputs are host-preshuffled to partition-major so startup loads
are descriptor-cheap; ma zero rows come from a DRAM zeros tensor (no DVE
memsets); two warmup matmul bursts hold the PE HAM clock at 8/8.

Sharding: data-parallel over batch (B=8 -> 8 cores), no collectives.
"""

import numpy as np
import sys

sys.path.insert(0, "/opt/trn_rl_repo")

B, C, N, T = 8, 1, 1024, 64
H = 64
NT = N * T          # 65536 per-core output elements
CHUNK = 512
NPAIR = 64          # pairs (g, g+64); chunk c=g is lane0, c=g+64 lane1
HALF = NT // 2      # 32768: lane1 position offset
BLK = 4096          # ma tile columns (64 nodes x 64 steps)
NBLK = 8            # blocks per lane

_CACHE = {}


def _fold_weights(W_fs, b_fs, W_in, b_in, W_gc, b_gc, W_lo, b_lo, prelu_a,
                  W_ro, b_ro, W_o1, b_o1, W_o2, b_o2, adj):
    """Host-side weight folding in float64 for accuracy."""
    f8 = np.float64
    W_in, b_in = W_in.astype(f8), b_in.astype(f8)
    W_gc, b_gc = W_gc.astype(f8), b_gc.astype(f8)
    W_lo, b_lo = W_lo.astype(f8), b_lo.astype(f8)
    W_ro, b_ro = W_ro.astype(f8), b_ro.astype(f8)
    W_o1, b_o1 = W_o1.astype(f8), b_o1.astype(f8)
    W_o2, b_o2 = W_o2.astype(f8), b_o2.astype(f8)

    W0 = W_in[:, 0]           # x1 channel  [64]
    W1 = W_in[:, 1]           # mask channel [64]
    Wlo1 = W_lo[:, :H]
    M1 = Wlo1 @ W_gc[:, :H]
    M2 = Wlo1 @ W_gc[:, H:]
    b_fold = Wlo1 @ b_gc + b_lo

    b_in_p = b_in + W0 * float(b_fs[0])
    PA = np.stack([
        M1 @ W0,
        M1 @ W1,
        M2 @ W0,
        M2 @ W1,
        M2 @ b_in_p,
    ])                                     # [5, 64]  lhsT for pass A
    bias_v = M1 @ b_in_p + b_fold          # [64] -> Prelu bias

    w_ro1 = W_ro[0, :H]                    # [64]
    PB = np.outer(w_ro1, W_o1[:, 0])       # [64(h), 64(f)] lhsT for pass B
    bias_f = W_o1[:, 0] * b_ro[0] + b_o1   # [64]

    den = float(np.sum(W_o2[0] ** 2))
    assert abs(den) > 1e-12
    k = float(b_o2[0]) * W_o2[0] / den     # fold b_o2: W_o2.(rr + k) adds b_o2

    cg = adj.astype(f8).sum(axis=0)        # [N] column sums of adj
    cgrep = np.repeat(cg, T)               # [(n,t)] layout n*T + t

    a = float(prelu_a)
    assert 0.0 < a < 1.0

    # pass A stationary: lane0 streams at rows 0:5 -> out 0:64,
    #                    lane1 streams at rows 32:37 -> out 64:128
    paA = np.zeros((128, 128))
    paA[0:5, 0:64] = PA
    paA[32:37, 64:128] = PA
    # pass B stationary: blockdiag(PB, PB) over packed prelu [128, 512]
    pbB = np.zeros((128, 128))
    pbB[0:64, 0:64] = PB
    pbB[64:128, 64:128] = PB
    # pass C stationary: col 0 <- lane0 W_o2, col 32 <- lane1 W_o2
    pcC = np.zeros((128, 64))
    pcC[0:64, 0] = W_o2[0]
    pcC[64:128, 32] = W_o2[0]

    h16 = np.float16
    fp = np.float32
    ch = np.zeros((128, 320))
    ch[:, 0:128] = paA
    ch[:, 128:256] = pbB
    ch[:, 256:320] = pcC
    cf = np.zeros((128, 4))
    cf[0:64, 0] = bias_f + k
    cf[64:128, 0] = bias_f + k
    cf[0:64, 1] = k
    cf[64:128, 1] = k
    cf[:, 2] = b_fs[0]
    cf[0:64, 3] = bias_v
    cf[64:128, 3] = bias_v

    return dict(
        consts_h=ch.astype(h16),
        consts_f=cf.astype(fp),
        cgrep=cgrep.astype(h16),
        prelu_a=a,
    )


def _build_program(prelu_a):
    import concourse.bass as bass
    import concourse.bacc as bacc
    import concourse.mybir as mybir
    import concourse.tile as tile

    dt = mybir.dt
    f32 = dt.float32
    h16 = dt.float16
    AF = mybir.ActivationFunctionType
    ALU = mybir.AluOpType

    nc = bacc.Bacc("TRN2", target_bir_lowering=False, debug=False,
                   num_devices=B)

    # host-preshuffled: xs[p, nt*64+t] = x[nt*128+p, t]; ms likewise;
    # adjs[p, nt*1024+c] = adj[nt*128+p, c]  (descriptor-efficient loads)
    xs_d = nc.dram_tensor("xs", [128, 512], h16, kind="ExternalInput")
    ms_d = nc.dram_tensor("ms", [128, 512], h16, kind="ExternalInput")
    zr = nc.dram_tensor("zr", [91, BLK], h16, kind="ExternalInput")
    adjs = nc.dram_tensor("adjs", [128, 8192], h16, kind="ExternalInput")
    cgrep = nc.dram_tensor("cgrep", [NT], h16, kind="ExternalInput")
    ch_d = nc.dram_tensor("consts_h", [128, 320], h16, kind="ExternalInput")
    cf_d = nc.dram_tensor("consts_f", [128, 4], f32, kind="ExternalInput")
    out_d = nc.dram_tensor("out", [NT], h16, kind="ExternalOutput")

    MOVA_BUFS = 4
    from contextlib import ExitStack
    with tile.TileContext(nc) as tc, ExitStack() as ctx:
        const = ctx.enter_context(tc.tile_pool(name="const", bufs=1))
        adjp = ctx.enter_context(tc.tile_pool(name="adjp", bufs=1))
        gmovp = ctx.enter_context(tc.tile_pool(name="gmovp", bufs=1))
        movap = ctx.enter_context(tc.tile_pool(name="movap", bufs=1))
        p2p = ctx.enter_context(tc.tile_pool(name="p2p", bufs=4))
        rrp = ctx.enter_context(tc.tile_pool(name="rrp", bufs=4))
        posbp = ctx.enter_context(tc.tile_pool(name="posbp", bufs=3))
        vps = ctx.enter_context(tc.tile_pool(name="vps", bufs=2, space="PSUM"))
        wps = ctx.enter_context(tc.tile_pool(name="wps", bufs=2, space="PSUM"))
        pop = ctx.enter_context(tc.tile_pool(name="pop", bufs=2, space="PSUM"))
        gps = ctx.enter_context(tc.tile_pool(name="gps", bufs=2, space="PSUM"))

        # ---- consolidated constant loads (2 DMAs) ---------------------
        ch_t = const.tile([128, 320], h16)
        cf_t = const.tile([128, 4], f32)
        nc.sync.dma_start(out=ch_t[:], in_=ch_d[:])
        nc.sync.dma_start(out=cf_t[:], in_=cf_d[:])
        pa_t = ch_t[:, 0:128]
        pb_t = ch_t[:, 128:256]
        pc_t = ch_t[:, 256:320]
        bk_t = cf_t[:, 0:1]
        kk_t = cf_t[:, 1:2]
        bfs_t = cf_t[:, 2:3]
        bv_t = cf_t[:, 3:4]

        # ---- ma tiles: 4 rotating [128, 4096] fp16 --------------------
        # rows 0:5   = lane0 streams (u, m, ug, mg, cg)
        # rows 32:37 = lane1 streams (nodes 512:1024)
        # other rows zeroed from the zr dram tensor (no DVE memsets)
        ma4 = [movap.tile([128, BLK], h16, tag=f"mova{i}", name=f"mova{i}")
               for i in range(MOVA_BUFS)]
        zq = [nc.sync, nc.gpsimd, nc.scalar]
        for i in range(MOVA_BUFS):
            zq[i % 3].dma_start(out=ma4[i][5:32, :], in_=zr[0:27, :])
            zq[(i + 1) % 3].dma_start(out=ma4[i][37:128, :], in_=zr[:, :])

        # ---- x/m: contiguous loads into the combined gmov tile --------
        # gmov[p, nt*64 + t]       = u[node nt*128+p, t]  (after STT)
        # gmov[p, 512 + nt*64 + t] = m[node nt*128+p, t]
        gmov = gmovp.tile([128, 1024], h16, tag="gmov", name="gmov")
        nc.sync.dma_start(out=gmov[:, 0:512], in_=xs_d[:, :])
        nc.sync.dma_start(out=gmov[:, 512:1024], in_=ms_d[:, :])
        # u = (x - bfs)*m  (one contiguous op; +bfs folded into weights)
        gx_cols = gmov[:, 0:512]
        gm_cols = gmov[:, 512:1024]
        nc.vector.scalar_tensor_tensor(
            out=gx_cols, in0=gx_cols, scalar=bfs_t,
            in1=gm_cols, op0=ALU.subtract, op1=ALU.mult)
        # G-pass moving operand for node-tile nt: [u(64 t) | m(64 t)]
        gmh = gmov[:, :].rearrange("p (h q) -> p h q", h=2)

        # ---- adj: 3 contiguous DMAs off the sync queue ----------------
        adjt = adjp.tile([128, 8192], h16, tag="adjt", name="adjt")
        nc.gpsimd.dma_start(out=adjt[:, 0:3072], in_=adjs[:, 0:3072])
        nc.scalar.dma_start(out=adjt[:, 3072:6144], in_=adjs[:, 3072:6144])
        nc.scalar.dma_start(out=adjt[:, 6144:8192], in_=adjs[:, 6144:8192])

        # ---- HAM warmup: keep PE streaming so the clock hits 8/8 ------
        warm_ps = gps.tile([128, 512], f32, tag="psg", name="warm")
        for wi in range(25):
            nc.tensor.matmul(warm_ps[:, 0:128], pa_t, pb_t,
                             start=True, stop=True, skip_group_check=True)

        gx = [None] * 8
        ma_t = [None] * NBLK
        vps_t = {}
        wps_t = {}
        p2_t = {}
        rr_t = {}
        po_ps = {}

        def emit_g(mt):
            psg = gps.tile([128, 512], f32, tag="psg", name=f"psg{mt}")
            for nt in range(8):
                nc.tensor.matmul(
                    psg[:, 0:128],
                    adjt[:, nt * 1024 + mt * 128:nt * 1024 + (mt + 1) * 128],
                    gmh[:, :, nt * 64:(nt + 1) * 64],
                    start=(nt == 0), stop=(nt == 7))
            g = gmovp.tile([128, 128], h16, tag=f"gxm{mt}", name=f"gxm{mt}")
            nc.scalar.activation(g[:], psg[:, 0:128], AF.Copy,
                                 bias=0.0, scale=1.0)
            gx[mt] = g

        cg2 = cgrep[:].rearrange("(l q) -> l q", l=2)

        def emit_ma(blk):
            # lane0: nodes [blk*64, blk*64+64) -> gmov/gx block blk//2,
            #        half p0 = (blk%2)*64;  lane1: nodes +512
            mt0, mt1 = blk // 2, 4 + blk // 2
            p0 = (blk % 2) * 64
            ma = ma4[blk % MOVA_BUFS]
            c0, c1 = mt0 * 64, mt1 * 64
            nc.sync.dma_start(out=ma[0:1, :], in_=gmov[p0:p0 + 64, c0:c0 + 64])
            nc.sync.dma_start(out=ma[1:2, :],
                              in_=gmov[p0:p0 + 64, 512 + c0:512 + c0 + 64])
            nc.gpsimd.dma_start(out=ma[2:3, :], in_=gx[mt0][p0:p0 + 64, 0:64])
            nc.sync.dma_start(out=ma[3:4, :], in_=gx[mt0][p0:p0 + 64, 64:128])
            nc.gpsimd.dma_start(out=ma[4:5, :],
                                in_=cg2[0:1, blk * BLK:(blk + 1) * BLK])
            nc.gpsimd.dma_start(out=ma[32:33, :],
                                in_=gmov[p0:p0 + 64, c1:c1 + 64])
            nc.sync.dma_start(out=ma[33:34, :],
                              in_=gmov[p0:p0 + 64, 512 + c1:512 + c1 + 64])
            nc.gpsimd.dma_start(out=ma[34:35, :], in_=gx[mt1][p0:p0 + 64, 0:64])
            nc.sync.dma_start(out=ma[35:36, :],
                              in_=gx[mt1][p0:p0 + 64, 64:128])
            nc.gpsimd.dma_start(out=ma[36:37, :],
                                in_=cg2[1:2, blk * BLK:(blk + 1) * BLK])
            ma_t[blk] = ma

        def st_a(g):
            blk, j = g // 8, g % 8
            if g == 0:
                emit_g(0)
                emit_g(4)
                emit_ma(0)
                # second warmup burst bridges the block-0 gather wait
                warm2 = gps.tile([128, 512], f32, tag="psg", name="warm2")
                for wi in range(25):
                    nc.tensor.matmul(warm2[:, 0:128], pa_t, pb_t,
                                     start=True, stop=True,
                                     skip_group_check=True)
            if j == 0 and blk + 1 < NBLK:
                if (blk + 1) % 2 == 0:
                    emit_g((blk + 1) // 2)
                    emit_g(4 + (blk + 1) // 2)
                emit_ma(blk + 1)
            c0 = j * CHUNK
            ps_v = vps.tile([128, 512], f32, tag="v", name=f"v{g}")
            nc.tensor.matmul(ps_v[:], pa_t, ma_t[blk][:, c0:c0 + CHUNK],
                             start=True, stop=True)
            p2 = p2p.tile([128, CHUNK], h16, tag="p2", name=f"p2{g}")
            nc.scalar.activation(p2[:], ps_v[:], AF.Prelu,
                                 bias=bv_t, scale=1.0, alpha=prelu_a)
            vps_t[g] = ps_v
            p2_t[g] = p2

        def st_b(g):
            del vps_t[g]
            ps_w = wps.tile([128, 512], f32, tag="w", name=f"w{g}")
            nc.tensor.matmul(ps_w[:], pb_t, p2_t[g][:],
                             start=True, stop=True)
            del p2_t[g]
            rr = rrp.tile([128, CHUNK], h16, tag="rr", name=f"rr{g}")
            nc.vector.tensor_scalar(
                out=rr[:, :], in0=ps_w[:, :],
                scalar1=bk_t, scalar2=kk_t,
                op0=ALU.add, op1=ALU.max)
            wps_t[g] = ps_w
            rr_t[g] = rr

        def st_c(g):
            del wps_t[g]
            q = g % 2
            if q == 0:
                po_ps[g // 2] = pop.tile([128, 512], f32, tag="po",
                                         name=f"po{g // 2}")
            # pair g outputs: row 64q = lane0 (chunk g), row 64q+32 = lane1
            nc.tensor.matmul(po_ps[g // 2][64 * q:64 * q + 64, :], pc_t,
                             rr_t[g][:, :], start=True, stop=True,
                             tile_position=(0, 64 * q))
            del rr_t[g]
            if q == 1:
                bi = g // 2
                po_sb = posbp.tile([97, 512], h16, tag="po_sb",
                                   name=f"po_sb{bi}")
                if bi % 2 == 0:
                    nc.scalar.activation(po_sb[0:97, :], po_ps[bi][0:97, :],
                                         AF.Identity, bias=0.0, scale=1.0)
                else:
                    nc.vector.tensor_copy(po_sb[0:97, :], po_ps[bi][0:97, :])
                del po_ps[bi]
                # rows {0,64} = lane0 chunks (g-1, g) -> out[(g-1)*512 ...)
                # rows {32,96} = lane1 chunks (+64)  -> out[HALF + ...]
                o0 = (g - 1) * CHUNK
                nc.sync.dma_start(out=out_d[o0:o0 + 2 * CHUNK],
                                  in_=po_sb[0:65:64, :])
                nc.gpsimd.dma_start(out=out_d[HALF + o0:HALF + o0 + 2 * CHUNK],
                                    in_=po_sb[32:97:64, :])

        for p in range(NPAIR + 4):
            if p < NPAIR:
                st_a(p)
            if 2 <= p < NPAIR + 2:
                st_b(p - 2)
            if p >= 4:
                st_c(p - 4)

    nc.compile()
    return nc


def _get_program(prelu_a):
    key = ("prog", float(prelu_a))
    if key not in _CACHE:
        _CACHE[key] = _build_program(prelu_a)
    return _CACHE[key]


def make_in_maps(x, mask, W_fs, b_fs, W_in, b_in, adj, W_gc, b_gc, W_lo, b_lo,
                 prelu_a, W_ro, b_ro, W_o1, b_o1, W_o2, b_o2):
    x = np.asarray(x, np.float32)
    mask_f = np.asarray(mask, np.float16)
    adj = np.asarray(adj, np.float32)

    folded = _fold_weights(np.asarray(W_fs), np.asarray(b_fs),
                           np.asarray(W_in), np.asarray(b_in),
                           np.asarray(W_gc), np.asarray(b_gc),
                           np.asarray(W_lo), np.asarray(b_lo),
                           float(prelu_a),
                           np.asarray(W_ro), np.asarray(b_ro),
                           np.asarray(W_o1), np.asarray(b_o1),
                           np.asarray(W_o2), np.asarray(b_o2), adj)

    adjs = np.ascontiguousarray(
        adj.astype(np.float16).reshape(8, 128, N).transpose(1, 0, 2)
    ).reshape(128, 8 * N)
    shared = dict(adjs=adjs, cgrep=folded["cgrep"],
                  consts_h=folded["consts_h"], consts_f=folded["consts_f"],
                  zr=np.zeros((91, BLK), np.float16))
    in_maps = []
    for b in range(B):
        m = dict(shared)
        xh = x[b, 0].astype(np.float16)
        mh = mask_f[b, 0]
        m["xs"] = np.ascontiguousarray(
            xh.reshape(8, 128, T).transpose(1, 0, 2)).reshape(128, 8 * T)
        m["ms"] = np.ascontiguousarray(
            mh.reshape(8, 128, T).transpose(1, 0, 2)).reshape(128, 8 * T)
        in_maps.append(m)
    return in_maps, folded["prelu_a"]


def kernel(x, mask, W_fs, b_fs, W_in, b_in, adj, W_gc, b_gc, W_lo, b_lo,
           prelu_a, W_ro, b_ro, W_o1, b_o1, W_o2, b_o2):
    in_maps, a = make_in_maps(x, mask, W_fs, b_fs, W_in, b_in, adj, W_gc,
                              b_gc, W_lo, b_lo, prelu_a, W_ro, b_ro, W_o1,
                              b_o1, W_o2, b_o2)
    nc = _get_program(a)

    from concourse.bass_utils import run_bass_kernel_spmd
    res = run_bass_kernel_spmd(nc, in_maps, list(range(B)))

    out = np.empty((B, C, N, T), np.float32)
    for b in range(B):
        out[b, 0] = np.asarray(res.results[b]["out"]).reshape(N, T)
    return out  # fp16 device output upcast to f32 on assignment
